# revision 2
# baseline (speedup 1.0000x reference)
"""Cross-attention kernel for Trainium2, data-parallel over batch on 8 cores.

Per core (one batch element):
  Q = x @ Wq + bq ; K = e @ Wk + bk ; V = e @ Wv + bv
  out = softmax(Q K^T / 8) @ V

Layout strategy (everything oriented so the contraction dim sits on SBUF
partitions, avoiding all transposes except one PE-transpose of x and e):
  xT, eT  [d_in, s]   fp16   (PE transpose of the fp32 inputs, cast on evict)
  QT, KT  [d_out, s]  fp16   = W^T @ xT   (weights stationary, bias via ACT)
  V       [skv, d]    bf16   = eT^T @ Wv  (bias via ones-row outer product)
  per 512-wide sq strip:
    ST    [skv, 512]  psum   = KT^T @ QT
    PT    [skv, 512]  bf16   = exp(ST/8)  (no max subtraction: |s/8| < ~25,
                                           safe in fp32 psum / bf16 storage)
    out   [sq, d]     f32    = (PT^T @ V) * 1/(PT^T @ 1)  (row sums from the
                                           same stationary operand, N=1 matmul)

Numerics: fp16 projections + bf16 probs/V gives ~2e-3 rel L2 error vs the
fp32 reference (bf16 everywhere would be ~1e-2).
"""

import numpy as np

import concourse.bacc as bacc
import concourse.bass as bass
import concourse.mybir as mybir
import concourse.tile as tile
from concourse.bass_utils import run_bass_kernel_spmd
from concourse.masks import make_identity

P = 128
D = 1024
ND = D // P  # 8 d tiles
SQ = 2048
NSQ = SQ // P  # 16
SKV = 2048
NSKV = SKV // P  # 16
NC = SQ // 512  # 4 strips of 512 along s
N_CORES = 8

F32 = mybir.dt.float32
F16 = mybir.dt.float16
BF16 = mybir.dt.bfloat16
AF = mybir.ActivationFunctionType


def _load_transpose(nc, ld_pool, ps_tr, dst_tiles, src_dram, identity):
    """Load a [2048, 1024] f32 DRAM tensor and produce its fp16 transpose
    as 8 SBUF tiles [128 (d), 2048 (s)]."""
    loads = []
    for st in range(NSQ):
        ldt = ld_pool.tile([P, D], F32, name=f"ld_{src_dram.name}_{st}", tag="ld")
        nc.sync.dma_start(ldt[:], src_dram.ap()[st * P : (st + 1) * P, :])
        loads.append(ldt)
    for sg in range(NC):  # groups of 4 s-tiles -> one [128, 512] psum bank
        for dit in range(ND):
            pst = ps_tr.tile([P, 512], F32, name=f"pst_{src_dram.name}", tag="pst")
            for j in range(4):
                nc.tensor.matmul(
                    pst[:, j * P : (j + 1) * P],
                    loads[sg * 4 + j][:, dit * P : (dit + 1) * P],
                    identity[:],
                    is_transpose=True,
                    start=(j == 0),
                    stop=(j == 3),
                )
            nc.vector.tensor_copy(
                dst_tiles[dit][:, sg * 512 : (sg + 1) * 512], pst[:]
            )


def _load_w16(nc, ld_pool, w16_pool, w_dram):
    tiles = []
    for dit in range(ND):
        wl = ld_pool.tile([P, D], F32, name=f"wl_{w_dram.name}_{dit}", tag="ld")
        nc.sync.dma_start(wl[:], w_dram.ap()[dit * P : (dit + 1) * P, :])
        w16t = w16_pool.tile([P, D], F16, name=f"w16_{w_dram.name}_{dit}", tag="w16")
        nc.vector.tensor_copy(w16t[:], wl[:])
        tiles.append(w16t)
    return tiles


def _project_t(nc, ps_proj, w16, src_t, dst_tiles, bias_cols):
    """dst (f16, [d_out, s] as 8 tiles) = w^T @ src_t + bias (per-partition)."""
    for c in range(NC):
        for dot in range(ND):
            psq = ps_proj.tile([P, 512], F32, name="psq", tag="psp")
            for dit in range(ND):
                nc.tensor.matmul(
                    psq[:],
                    w16[dit][:, dot * P : (dot + 1) * P],
                    src_t[dit][:, c * 512 : (c + 1) * 512],
                    start=(dit == 0),
                    stop=(dit == ND - 1),
                )
            nc.scalar.activation(
                dst_tiles[dot][:, c * 512 : (c + 1) * 512],
                psq[:],
                AF.Identity,
                bias=bias_cols[:, dot : dot + 1],
            )


def build():
    nc = bacc.Bacc("TRN2", target_bir_lowering=False, debug=False)
    x = nc.declare_dram_parameter("x", [SQ, D], F32, isOutput=False)
    e = nc.declare_dram_parameter("e", [SKV, D], F32, isOutput=False)
    wq = nc.declare_dram_parameter("wq", [D, D], F32, isOutput=False)
    wk = nc.declare_dram_parameter("wk", [D, D], F32, isOutput=False)
    wv = nc.declare_dram_parameter("wv", [D, D], F32, isOutput=False)
    bq = nc.declare_dram_parameter("bq", [D], F32, isOutput=False)
    bk = nc.declare_dram_parameter("bk", [D], F32, isOutput=False)
    bv = nc.declare_dram_parameter("bv", [D], F32, isOutput=False)
    out = nc.declare_dram_parameter("out", [SQ, D], F32, isOutput=True)

    with tile.TileContext(nc) as tc:
        # ---- long-lived pools (left stack, released in LIFO order) ----
        const = tc.alloc_tile_pool(name="const", bufs=1, side="left")
        qt_pool = tc.alloc_tile_pool(name="qt", bufs=ND, side="left")
        kt_pool = tc.alloc_tile_pool(name="kt", bufs=ND, side="left")
        w16_pool = tc.alloc_tile_pool(name="w16", bufs=16, side="left")
        et_pool = tc.alloc_tile_pool(name="et", bufs=ND, side="left")
        xt_pool = tc.alloc_tile_pool(name="xt", bufs=ND, side="left")
        ldA = tc.alloc_tile_pool(name="ldA", bufs=6, side="left")
        ps_proj = tc.alloc_tile_pool(name="ps_proj", bufs=4, space="PSUM")
        ps_tr = tc.alloc_tile_pool(name="ps_tr", bufs=4, space="PSUM")

        identity = const.tile([P, P], F32, tag="ident")
        make_identity(nc, identity[:])
        ones_row = const.tile([1, P], F16, tag="ones_row")
        nc.gpsimd.memset(ones_row[:], 1.0)
        ones_col = const.tile([P, 1], BF16, tag="ones_col")
        nc.gpsimd.memset(ones_col[:], 1.0)
        bqt = const.tile([P, ND], F32, tag="bqt")
        nc.sync.dma_start(bqt[:], bq.ap().rearrange("(t p) -> p t", p=P))
        bkt = const.tile([P, ND], F32, tag="bkt")
        nc.sync.dma_start(bkt[:], bk.ap().rearrange("(t p) -> p t", p=P))
        bvl = ldA.tile([1, D], F32, tag="ld")
        nc.sync.dma_start(bvl[:], bv.ap().rearrange("(a n) -> a n", a=1))
        bv16 = const.tile([1, D], F16, tag="bv16")
        nc.vector.tensor_copy(bv16[:], bvl[:])

        # ---- x -> xT ; Wq ; QT ----
        xT = [xt_pool.tile([P, SQ], F16, name=f"xT{d}", tag="xT") for d in range(ND)]
        _load_transpose(nc, ldA, ps_tr, xT, x, identity)
        wq16 = _load_w16(nc, ldA, w16_pool, wq)
        ldA.release()

        qT = [qt_pool.tile([P, SQ], F16, name=f"qT{d}", tag="qT") for d in range(ND)]
        _project_t(nc, ps_proj, wq16, xT, qT, bqt)
        xt_pool.release()

        # ---- e -> eT ; Wk ; KT ----
        ldB = tc.alloc_tile_pool(name="ldB", bufs=6, side="left")
        eT = [et_pool.tile([P, SKV], F16, name=f"eT{d}", tag="eT") for d in range(ND)]
        _load_transpose(nc, ldB, ps_tr, eT, e, identity)
        wk16 = _load_w16(nc, ldB, w16_pool, wk)
        ldB.release()

        kT = [kt_pool.tile([P, SKV], F16, name=f"kT{d}", tag="kT") for d in range(ND)]
        _project_t(nc, ps_proj, wk16, eT, kT, bkt)
        ps_tr.release()

        # ---- Wv ; V ----
        ldC = tc.alloc_tile_pool(name="ldC", bufs=4, side="left")
        wv16 = _load_w16(nc, ldC, w16_pool, wv)
        ldC.release()

        v_pool = tc.alloc_tile_pool(name="v", bufs=NSKV, side="right")
        vt = [v_pool.tile([P, D], BF16, name=f"v{t}", tag="v") for t in range(NSKV)]
        for kt_i in range(NSKV):
            ps_half = []
            for h in range(2):
                psv = ps_proj.tile([P, 512], F32, name=f"psv{h}", tag="psp")
                nc.tensor.matmul(
                    psv[:],
                    ones_row[:],
                    bv16[:, h * 512 : (h + 1) * 512],
                    start=True,
                    stop=False,
                )
                ps_half.append(psv)
            for dit in range(ND):
                for h in range(2):
                    nc.tensor.matmul(
                        ps_half[h][:],
                        eT[dit][:, kt_i * P : (kt_i + 1) * P],
                        wv16[dit][:, h * 512 : (h + 1) * 512],
                        start=False,
                        stop=(dit == ND - 1),
                    )
            for h in range(2):
                nc.vector.tensor_copy(
                    vt[kt_i][:, h * 512 : (h + 1) * 512], ps_half[h][:]
                )
        ps_proj.release()
        et_pool.release()
        w16_pool.release()

        # ---- attention, strip-wise over sq ----
        ptS_pool = tc.alloc_tile_pool(name="ptS", bufs=2 * NSKV, side="right")
        outp = tc.alloc_tile_pool(name="outp", bufs=4, side="right")
        small = tc.alloc_tile_pool(name="small", bufs=4, side="right")
        ps_st = tc.alloc_tile_pool(name="ps_st", bufs=2, space="PSUM")
        ps_pv = tc.alloc_tile_pool(name="ps_pv", bufs=2, space="PSUM")
        ps_sum = tc.alloc_tile_pool(name="ps_sum", bufs=2, space="PSUM")

        for strip in range(NC):
            s0 = strip * 512
            ptS = [
                ptS_pool.tile([P, 512], BF16, name=f"ptS_{strip}_{t}", tag="ptS")
                for t in range(NSKV)
            ]
            for kt_i in range(NSKV):
                pss = ps_st.tile([P, 512], F32, name="pss_st", tag="pss_st")
                for dit in range(ND):
                    nc.tensor.matmul(
                        pss[:],
                        kT[dit][:, kt_i * P : (kt_i + 1) * P],
                        qT[dit][:, s0 : s0 + 512],
                        start=(dit == 0),
                        stop=(dit == ND - 1),
                    )
                nc.scalar.activation(ptS[kt_i][:], pss[:], AF.Exp, scale=0.125)
            for j in range(4):
                sqt = strip * 4 + j
                pso = ps_pv.tile([P, D], F32, name="pso", tag="pso")
                psum_s = ps_sum.tile([P, 1], F32, name="psum_s", tag="psum_s")
                for kt_i in range(NSKV):
                    lhsT = ptS[kt_i][:, j * P : (j + 1) * P]
                    first = kt_i == 0
                    last = kt_i == NSKV - 1
                    for h in range(2):
                        nc.tensor.matmul(
                            pso[:, h * 512 : (h + 1) * 512],
                            lhsT,
                            vt[kt_i][:, h * 512 : (h + 1) * 512],
                            start=first,
                            stop=last,
                        )
                    nc.tensor.matmul(
                        psum_s[:], lhsT, ones_col[:], start=first, stop=last
                    )
                recip = small.tile([P, 1], F32, name="recip", tag="recip")
                nc.vector.reciprocal(recip[:], psum_s[:])
                ot = outp.tile([P, D], F32, name="ot", tag="ot")
                nc.vector.tensor_scalar_mul(ot[:], pso[:], recip[:])
                nc.sync.dma_start(out.ap()[sqt * P : (sqt + 1) * P, :], ot[:])

        ps_sum.release()
        ps_pv.release()
        ps_st.release()
        small.release()
        outp.release()
        ptS_pool.release()
        v_pool.release()
        kt_pool.release()
        qt_pool.release()
        const.release()

    nc.compile()
    return nc


_NC_CACHE = []


def _get_nc():
    if not _NC_CACHE:
        _NC_CACHE.append(build())
    return _NC_CACHE[0]


def kernel(
    hidden_states,
    encoder_hidden_states,
    Wq,
    bq,
    Wk,
    bk,
    Wv,
    bv,
    _trace=False,
    _trace_kwargs=None,
):
    hs = np.ascontiguousarray(np.asarray(hidden_states, np.float32))
    es = np.ascontiguousarray(np.asarray(encoder_hidden_states, np.float32))
    wq_ = np.ascontiguousarray(np.asarray(Wq, np.float32))
    wk_ = np.ascontiguousarray(np.asarray(Wk, np.float32))
    wv_ = np.ascontiguousarray(np.asarray(Wv, np.float32))
    bq_ = np.ascontiguousarray(np.asarray(bq, np.float32))
    bk_ = np.ascontiguousarray(np.asarray(bk, np.float32))
    bv_ = np.ascontiguousarray(np.asarray(bv, np.float32))

    nc = _get_nc()
    in_maps = [
        {
            "x": hs[c],
            "e": es[c],
            "wq": wq_,
            "wk": wk_,
            "wv": wv_,
            "bq": bq_,
            "bk": bk_,
            "bv": bv_,
        }
        for c in range(N_CORES)
    ]
    res = run_bass_kernel_spmd(
        nc,
        in_maps,
        list(range(N_CORES)),
        trace=_trace,
        **(_trace_kwargs or {}),
    )
    out = np.stack([res.results[c]["out"] for c in range(N_CORES)], axis=0)
    if _trace:
        return out, res
    return out


# revision 4
# speedup vs baseline: 10.0698x; 10.0698x over previous
"""Cross-attention kernel for Trainium2, data-parallel over batch on 8 cores.

Per core (one batch element):
  Q = x @ Wq + bq ; K = e @ Wk + bk ; V = e @ Wv + bv
  out = softmax(Q K^T / 8) @ V

Layout strategy (everything oriented so the contraction dim sits on SBUF
partitions, avoiding all transposes except one PE-transpose of x and e):
  xT, eT  [d_in, s]   fp16   (PE transpose of the fp32 inputs, cast on evict)
  QT, KT  [d_out, s]  fp16   = W^T @ xT   (weights stationary, bias via ACT)
  V       [skv, d]    bf16   = eT^T @ Wv  (bias via ones-row outer product)
  per 512-wide sq strip:
    ST    [skv, 512]  psum   = KT^T @ QT
    PT    [skv, 512]  bf16   = exp(ST/8)  (no max subtraction: |s/8| < ~25,
                                           safe in fp32 psum / bf16 storage)
    out   [sq, d]     f32    = (PT^T @ V) * 1/(PT^T @ 1)  (row sums from the
                                           same stationary operand, N=1 matmul)

Numerics: fp16 projections + bf16 probs/V gives ~2e-3 rel L2 error vs the
fp32 reference (bf16 everywhere would be ~1e-2).
"""

import numpy as np

import concourse.bacc as bacc
import concourse.bass as bass
import concourse.mybir as mybir
import concourse.tile as tile
from concourse.bass_utils import run_bass_kernel_spmd
from concourse.masks import make_identity

P = 128
D = 1024
ND = D // P  # 8 d tiles
SQ = 2048
NSQ = SQ // P  # 16
SKV = 2048
NSKV = SKV // P  # 16
NC = SQ // 512  # 4 strips of 512 along s
N_CORES = 8

F32 = mybir.dt.float32
F16 = mybir.dt.float16
BF16 = mybir.dt.bfloat16
AF = mybir.ActivationFunctionType


def _load_transpose(nc, ld_pool, ps_tr, dst_tiles, src_dram, identity):
    """Load a [2048, 1024] f32 DRAM tensor and produce its fp16 transpose
    as 8 SBUF tiles [128 (d), 2048 (s)]."""
    loads = []
    for st in range(NSQ):
        ldt = ld_pool.tile([P, D], F32, name=f"ld_{src_dram.name}_{st}", tag="ld")
        nc.sync.dma_start(ldt[:], src_dram.ap()[st * P : (st + 1) * P, :])
        loads.append(ldt)
    for sg in range(NC):  # groups of 4 s-tiles -> one [128, 512] psum bank
        for dit in range(ND):
            pst = ps_tr.tile([P, 512], F32, name=f"pst_{src_dram.name}", tag="pst")
            for j in range(4):
                nc.tensor.matmul(
                    pst[:, j * P : (j + 1) * P],
                    loads[sg * 4 + j][:, dit * P : (dit + 1) * P],
                    identity[:],
                    is_transpose=True,
                    start=(j == 0),
                    stop=(j == 3),
                )
            nc.vector.tensor_copy(
                dst_tiles[dit][:, sg * 512 : (sg + 1) * 512], pst[:]
            )


def _load_w16(nc, ld_pool, w16_pool, w_dram):
    tiles = []
    for dit in range(ND):
        wl = ld_pool.tile([P, D], F32, name=f"wl_{w_dram.name}_{dit}", tag="ld")
        nc.sync.dma_start(wl[:], w_dram.ap()[dit * P : (dit + 1) * P, :])
        w16t = w16_pool.tile([P, D], F16, name=f"w16_{w_dram.name}_{dit}", tag="w16")
        nc.vector.tensor_copy(w16t[:], wl[:])
        tiles.append(w16t)
    return tiles


def _project_t(nc, ps_proj, w16, src_t, dst_tiles, bias_cols):
    """dst (f16, [d_out, s] as 8 tiles) = w^T @ src_t + bias (per-partition)."""
    for c in range(NC):
        for dot in range(ND):
            psq = ps_proj.tile([P, 512], F32, name="psq", tag="psp")
            for dit in range(ND):
                nc.tensor.matmul(
                    psq[:],
                    w16[dit][:, dot * P : (dot + 1) * P],
                    src_t[dit][:, c * 512 : (c + 1) * 512],
                    start=(dit == 0),
                    stop=(dit == ND - 1),
                )
            nc.scalar.activation(
                dst_tiles[dot][:, c * 512 : (c + 1) * 512],
                psq[:],
                AF.Identity,
                bias=bias_cols[:, dot : dot + 1],
            )


def build(reps=1):
    nc = bacc.Bacc("TRN2", target_bir_lowering=False, debug=False)
    x = nc.declare_dram_parameter("x", [SQ, D], F32, isOutput=False)
    e = nc.declare_dram_parameter("e", [SKV, D], F32, isOutput=False)
    wq = nc.declare_dram_parameter("wq", [D, D], F32, isOutput=False)
    wk = nc.declare_dram_parameter("wk", [D, D], F32, isOutput=False)
    wv = nc.declare_dram_parameter("wv", [D, D], F32, isOutput=False)
    bq = nc.declare_dram_parameter("bq", [D], F32, isOutput=False)
    bk = nc.declare_dram_parameter("bk", [D], F32, isOutput=False)
    bv = nc.declare_dram_parameter("bv", [D], F32, isOutput=False)
    out = nc.declare_dram_parameter("out", [SQ, D], F32, isOutput=True)

    with tile.TileContext(nc) as tc:
        for _rep in range(reps):
            _emit_body(nc, tc, x, e, wq, wk, wv, bq, bk, bv, out)

    nc.compile()
    return nc


def _emit_body(nc, tc, x, e, wq, wk, wv, bq, bk, bv, out):
    if True:
        # ---- long-lived pools (left stack, released in LIFO order) ----
        const = tc.alloc_tile_pool(name="const", bufs=1, side="left")
        qt_pool = tc.alloc_tile_pool(name="qt", bufs=ND, side="left")
        kt_pool = tc.alloc_tile_pool(name="kt", bufs=ND, side="left")
        w16_pool = tc.alloc_tile_pool(name="w16", bufs=16, side="left")
        et_pool = tc.alloc_tile_pool(name="et", bufs=ND, side="left")
        xt_pool = tc.alloc_tile_pool(name="xt", bufs=ND, side="left")
        ldA = tc.alloc_tile_pool(name="ldA", bufs=6, side="left")
        ps_proj = tc.alloc_tile_pool(name="ps_proj", bufs=4, space="PSUM")
        ps_tr = tc.alloc_tile_pool(name="ps_tr", bufs=4, space="PSUM")

        identity = const.tile([P, P], F32, tag="ident")
        make_identity(nc, identity[:])
        ones_row = const.tile([1, P], F16, tag="ones_row")
        nc.gpsimd.memset(ones_row[:], 1.0)
        ones_col = const.tile([P, 1], BF16, tag="ones_col")
        nc.gpsimd.memset(ones_col[:], 1.0)
        bqt = const.tile([P, ND], F32, tag="bqt")
        nc.sync.dma_start(bqt[:], bq.ap().rearrange("(t p) -> p t", p=P))
        bkt = const.tile([P, ND], F32, tag="bkt")
        nc.sync.dma_start(bkt[:], bk.ap().rearrange("(t p) -> p t", p=P))
        bvl = ldA.tile([1, D], F32, tag="ld")
        nc.sync.dma_start(bvl[:], bv.ap().rearrange("(a n) -> a n", a=1))
        bv16 = const.tile([1, D], F16, tag="bv16")
        nc.vector.tensor_copy(bv16[:], bvl[:])

        # ---- x -> xT ; Wq ; QT ----
        xT = [xt_pool.tile([P, SQ], F16, name=f"xT{d}", tag="xT") for d in range(ND)]
        _load_transpose(nc, ldA, ps_tr, xT, x, identity)
        wq16 = _load_w16(nc, ldA, w16_pool, wq)
        ldA.release()

        qT = [qt_pool.tile([P, SQ], F16, name=f"qT{d}", tag="qT") for d in range(ND)]
        _project_t(nc, ps_proj, wq16, xT, qT, bqt)
        xt_pool.release()

        # ---- e -> eT ; Wk ; KT ----
        ldB = tc.alloc_tile_pool(name="ldB", bufs=6, side="left")
        eT = [et_pool.tile([P, SKV], F16, name=f"eT{d}", tag="eT") for d in range(ND)]
        _load_transpose(nc, ldB, ps_tr, eT, e, identity)
        wk16 = _load_w16(nc, ldB, w16_pool, wk)
        ldB.release()

        kT = [kt_pool.tile([P, SKV], F16, name=f"kT{d}", tag="kT") for d in range(ND)]
        _project_t(nc, ps_proj, wk16, eT, kT, bkt)
        ps_tr.release()

        # ---- Wv ; V ----
        ldC = tc.alloc_tile_pool(name="ldC", bufs=4, side="left")
        wv16 = _load_w16(nc, ldC, w16_pool, wv)
        ldC.release()

        v_pool = tc.alloc_tile_pool(name="v", bufs=NSKV, side="right")
        vt = [v_pool.tile([P, D], BF16, name=f"v{t}", tag="v") for t in range(NSKV)]
        for kt_i in range(NSKV):
            ps_half = []
            for h in range(2):
                psv = ps_proj.tile([P, 512], F32, name=f"psv{h}", tag="psp")
                nc.tensor.matmul(
                    psv[:],
                    ones_row[:],
                    bv16[:, h * 512 : (h + 1) * 512],
                    start=True,
                    stop=False,
                )
                ps_half.append(psv)
            for dit in range(ND):
                for h in range(2):
                    nc.tensor.matmul(
                        ps_half[h][:],
                        eT[dit][:, kt_i * P : (kt_i + 1) * P],
                        wv16[dit][:, h * 512 : (h + 1) * 512],
                        start=False,
                        stop=(dit == ND - 1),
                    )
            for h in range(2):
                nc.vector.tensor_copy(
                    vt[kt_i][:, h * 512 : (h + 1) * 512], ps_half[h][:]
                )
        ps_proj.release()
        et_pool.release()
        w16_pool.release()

        # ---- attention, strip-wise over sq ----
        ptS_pool = tc.alloc_tile_pool(name="ptS", bufs=2 * NSKV, side="right")
        outp = tc.alloc_tile_pool(name="outp", bufs=4, side="right")
        small = tc.alloc_tile_pool(name="small", bufs=4, side="right")
        ps_st = tc.alloc_tile_pool(name="ps_st", bufs=2, space="PSUM")
        ps_pv = tc.alloc_tile_pool(name="ps_pv", bufs=2, space="PSUM")
        ps_sum = tc.alloc_tile_pool(name="ps_sum", bufs=2, space="PSUM")

        for strip in range(NC):
            s0 = strip * 512
            ptS = [
                ptS_pool.tile([P, 512], BF16, name=f"ptS_{strip}_{t}", tag="ptS")
                for t in range(NSKV)
            ]
            for kt_i in range(NSKV):
                pss = ps_st.tile([P, 512], F32, name="pss_st", tag="pss_st")
                for dit in range(ND):
                    nc.tensor.matmul(
                        pss[:],
                        kT[dit][:, kt_i * P : (kt_i + 1) * P],
                        qT[dit][:, s0 : s0 + 512],
                        start=(dit == 0),
                        stop=(dit == ND - 1),
                    )
                nc.scalar.activation(ptS[kt_i][:], pss[:], AF.Exp, scale=0.125)
            for j in range(4):
                sqt = strip * 4 + j
                pso = ps_pv.tile([P, D], F32, name="pso", tag="pso")
                psum_s = ps_sum.tile([P, 1], F32, name="psum_s", tag="psum_s")
                for kt_i in range(NSKV):
                    lhsT = ptS[kt_i][:, j * P : (j + 1) * P]
                    first = kt_i == 0
                    last = kt_i == NSKV - 1
                    for h in range(2):
                        nc.tensor.matmul(
                            pso[:, h * 512 : (h + 1) * 512],
                            lhsT,
                            vt[kt_i][:, h * 512 : (h + 1) * 512],
                            start=first,
                            stop=last,
                        )
                    nc.tensor.matmul(
                        psum_s[:], lhsT, ones_col[:], start=first, stop=last
                    )
                recip = small.tile([P, 1], F32, name="recip", tag="recip")
                nc.vector.reciprocal(recip[:], psum_s[:])
                ot = outp.tile([P, D], F32, name="ot", tag="ot")
                nc.vector.tensor_scalar_mul(ot[:], pso[:], recip[:])
                nc.sync.dma_start(out.ap()[sqt * P : (sqt + 1) * P, :], ot[:])

        ps_sum.release()
        ps_pv.release()
        ps_st.release()
        small.release()
        outp.release()
        ptS_pool.release()
        v_pool.release()
        kt_pool.release()
        qt_pool.release()
        const.release()


_NC_CACHE = []


def _get_nc():
    if not _NC_CACHE:
        _NC_CACHE.append(build())
    return _NC_CACHE[0]


def kernel(
    hidden_states,
    encoder_hidden_states,
    Wq,
    bq,
    Wk,
    bk,
    Wv,
    bv,
    _trace=False,
    _trace_kwargs=None,
):
    hs = np.ascontiguousarray(np.asarray(hidden_states, np.float32))
    es = np.ascontiguousarray(np.asarray(encoder_hidden_states, np.float32))
    wq_ = np.ascontiguousarray(np.asarray(Wq, np.float32))
    wk_ = np.ascontiguousarray(np.asarray(Wk, np.float32))
    wv_ = np.ascontiguousarray(np.asarray(Wv, np.float32))
    bq_ = np.ascontiguousarray(np.asarray(bq, np.float32))
    bk_ = np.ascontiguousarray(np.asarray(bk, np.float32))
    bv_ = np.ascontiguousarray(np.asarray(bv, np.float32))

    nc = _get_nc()
    in_maps = [
        {
            "x": hs[c],
            "e": es[c],
            "wq": wq_,
            "wk": wk_,
            "wv": wv_,
            "bq": bq_,
            "bk": bk_,
            "bv": bv_,
        }
        for c in range(N_CORES)
    ]
    res = run_bass_kernel_spmd(
        nc,
        in_maps,
        list(range(N_CORES)),
        trace=_trace,
        **(_trace_kwargs or {}),
    )
    out = np.stack([res.results[c]["out"] for c in range(N_CORES)], axis=0)
    if _trace:
        return out, res
    return out


# revision 7
# speedup vs baseline: 11.6914x; 1.1610x over previous
"""Cross-attention kernel for Trainium2, data-parallel over batch on 8 cores.

Per core (one batch element):
  Q = x @ Wq + bq ; K = e @ Wk + bk ; V = e @ Wv + bv
  out = softmax(Q K^T / 8) @ V

Layout strategy (everything oriented so the contraction dim sits on SBUF
partitions, avoiding all transposes except one PE-transpose of x and e):
  xT, eT  [d_in, s]   fp16   (PE transpose of the fp32 inputs, cast on evict)
  QT, KT  [d_out, s]  fp16   = W^T @ xT   (weights stationary, bias via ACT)
  V       [skv, d]    bf16   = eT^T @ Wv  (bias via ones-row outer product)
  per 512-wide sq strip:
    ST    [skv, 512]  psum   = KT^T @ QT
    PT    [skv, 512]  bf16   = exp(ST/8)  (no max subtraction: |s/8| < ~25,
                                           safe in fp32 psum / bf16 storage)
    out   [sq, d]     f32    = (PT^T @ V) * 1/(PT^T @ 1)  (row sums from the
                                           same stationary operand, N=1 matmul)

The load -> transpose -> project pipeline is interleaved per 512-wide chunk
so the PE never waits on bulk DMA.

Numerics: fp16 projections + bf16 probs/V gives ~2e-3 rel L2 error vs the
fp32 reference (bf16 everywhere would be ~1e-2).
"""

import numpy as np

import concourse.bacc as bacc
import concourse.bass as bass
import concourse.mybir as mybir
import concourse.tile as tile
from concourse.bass_utils import run_bass_kernel_spmd
from concourse.masks import make_identity

P = 128
D = 1024
ND = D // P  # 8 d tiles
SQ = 2048
NSQ = SQ // P  # 16
SKV = 2048
NSKV = SKV // P  # 16
NC = SQ // 512  # 4 strips of 512 along s
N_CORES = 8

F32 = mybir.dt.float32
F16 = mybir.dt.float16
BF16 = mybir.dt.bfloat16
AF = mybir.ActivationFunctionType


def _load_w16(nc, ld_pool, w16_pool, w_dram):
    tiles = []
    for dit in range(ND):
        wl = ld_pool.tile([P, D], F32, name=f"wl_{w_dram.name}_{dit}", tag="ldw")
        nc.sync.dma_start(wl[:], w_dram.ap()[dit * P : (dit + 1) * P, :])
        w16t = w16_pool.tile([P, D], F16, name=f"w16_{w_dram.name}_{dit}", tag="w16")
        nc.vector.tensor_copy(w16t[:], wl[:])
        tiles.append(w16t)
    return tiles


def build(reps=1):
    nc = bacc.Bacc("TRN2", target_bir_lowering=False, debug=False)
    x = nc.declare_dram_parameter("x", [SQ, D], F32, isOutput=False)
    e = nc.declare_dram_parameter("e", [SKV, D], F32, isOutput=False)
    wq = nc.declare_dram_parameter("wq", [D, D], F32, isOutput=False)
    wk = nc.declare_dram_parameter("wk", [D, D], F32, isOutput=False)
    wv = nc.declare_dram_parameter("wv", [D, D], F32, isOutput=False)
    bq = nc.declare_dram_parameter("bq", [D], F32, isOutput=False)
    bk = nc.declare_dram_parameter("bk", [D], F32, isOutput=False)
    bv = nc.declare_dram_parameter("bv", [D], F32, isOutput=False)
    out = nc.declare_dram_parameter("out", [SQ, D], F32, isOutput=True)

    with tile.TileContext(nc) as tc:
        for _rep in range(reps):
            _emit_body(nc, tc, x, e, wq, wk, wv, bq, bk, bv, out)

    nc.compile()
    return nc


def _emit_body(nc, tc, x, e, wq, wk, wv, bq, bk, bv, out):
    # ---- left-stack pools (released LIFO) ----
    const = tc.alloc_tile_pool(name="const", bufs=1, side="left")
    qt_pool = tc.alloc_tile_pool(name="qt", bufs=ND, side="left")
    kt_pool = tc.alloc_tile_pool(name="kt", bufs=ND, side="left")
    w16_pool = tc.alloc_tile_pool(name="w16", bufs=16, side="left")
    et_pool = tc.alloc_tile_pool(name="et", bufs=ND, side="left")
    ldW = tc.alloc_tile_pool(name="ldW", bufs=3, side="left")
    ldE = tc.alloc_tile_pool(name="ldE", bufs=4, side="left")
    xl16_pool = tc.alloc_tile_pool(name="xl16", bufs=4, side="left")
    ps_proj = tc.alloc_tile_pool(name="ps_proj", bufs=4, space="PSUM")
    ps_tr = tc.alloc_tile_pool(name="ps_tr", bufs=4, space="PSUM")

    identity = const.tile([P, P], F16, tag="ident")
    make_identity(nc, identity[:])
    ones_row = const.tile([1, P], F16, tag="ones_row")
    nc.gpsimd.memset(ones_row[:], 1.0)
    ones_col = const.tile([P, 1], BF16, tag="ones_col")
    nc.gpsimd.memset(ones_col[:], 1.0)
    bqt = const.tile([P, ND], F32, tag="bqt")
    nc.sync.dma_start(bqt[:], bq.ap().rearrange("(t p) -> p t", p=P))
    bkt = const.tile([P, ND], F32, tag="bkt")
    nc.sync.dma_start(bkt[:], bk.ap().rearrange("(t p) -> p t", p=P))
    bvl = ldW.tile([1, D], F32, tag="ldw")
    nc.sync.dma_start(bvl[:], bv.ap().rearrange("(a n) -> a n", a=1))
    bv16 = const.tile([1, D], F16, tag="bv16")
    nc.vector.tensor_copy(bv16[:], bvl[:])

    def transpose_group(ld_tiles, dst_write, tag):
        """Cast 4 loaded [128, 1024] f32 tiles to fp16, then PE-transpose
        (fp16, 1 cyc/row) into [d, 512] chunks; dst_write(dit, psum_ap)
        evicts each."""
        l16 = []
        for j in range(4):
            t16 = xl16_pool.tile([P, D], F16, name=f"l16_{tag}_{j}", tag="l16")
            nc.vector.tensor_copy(t16[:], ld_tiles[j][:])
            l16.append(t16)
        for dit in range(ND):
            pst = ps_tr.tile([P, 512], F16, name=f"pst_{tag}", tag="pst")
            for j in range(4):
                nc.tensor.matmul(
                    pst[:, j * P : (j + 1) * P],
                    l16[j][:, dit * P : (dit + 1) * P],
                    identity[:],
                    is_transpose=True,
                    start=(j == 0),
                    stop=(j == 3),
                )
            dst_write(dit, pst)

    def project_chunk(w16, rhs_of_dit, dst_tiles, bias_cols, c):
        for dot in range(ND):
            psq = ps_proj.tile([P, 512], F32, name="psq", tag="psp")
            for dit in range(ND):
                nc.tensor.matmul(
                    psq[:],
                    w16[dit][:, dot * P : (dot + 1) * P],
                    rhs_of_dit(dit),
                    start=(dit == 0),
                    stop=(dit == ND - 1),
                )
            nc.scalar.activation(
                dst_tiles[dot][:, c * 512 : (c + 1) * 512],
                psq[:],
                AF.Identity,
                bias=bias_cols[:, dot : dot + 1],
            )

    # ---- x -> xT chunks -> QT, interleaved per 512-chunk ----
    xtc_pool = tc.alloc_tile_pool(name="xtc", bufs=2 * ND, side="left")
    ldX = tc.alloc_tile_pool(name="ldX", bufs=4, side="left")

    def load_group(pool, src_dram, c, tag):
        tiles = []
        for j in range(4):
            st = c * 4 + j
            t = pool.tile([P, D], F32, name=f"{tag}{c}_{j}", tag=tag)
            nc.sync.dma_start(t[:], src_dram.ap()[st * P : (st + 1) * P, :])
            tiles.append(t)
        return tiles

    qT = [qt_pool.tile([P, SQ], F16, name=f"qT{d}", tag="qT") for d in range(ND)]
    xg = {0: load_group(ldX, x, 0, "ldx")}
    wq16 = _load_w16(nc, ldW, w16_pool, wq)
    # start the e-side DMA early so the e-phase never waits on loads
    eg = {0: load_group(ldE, e, 0, "lde")}
    for c in range(NC):
        if c + 1 < NC:
            xg[c + 1] = load_group(ldX, x, c + 1, "ldx")
        xtc = [
            xtc_pool.tile([P, 512], F16, name=f"xtc{c}_{d}", tag="xtc")
            for d in range(ND)
        ]

        def wr_x(dit, pst, xtc=xtc):
            nc.vector.tensor_copy(xtc[dit][:], pst[:])

        transpose_group(xg.pop(c), wr_x, "x")
        project_chunk(wq16, lambda dit, xtc=xtc: xtc[dit][:], qT, bqt, c)
    ldX.release()
    xtc_pool.release()

    # ---- e -> eT (kept resident) -> KT, interleaved per 512-chunk ----
    eT = [et_pool.tile([P, SKV], F16, name=f"eT{d}", tag="eT") for d in range(ND)]
    kT = [kt_pool.tile([P, SKV], F16, name=f"kT{d}", tag="kT") for d in range(ND)]
    wk16 = _load_w16(nc, ldW, w16_pool, wk)
    for c in range(NC):
        if c + 1 < NC:
            eg[c + 1] = load_group(ldE, e, c + 1, "lde")

        def wr_e(dit, pst, c=c):
            nc.vector.tensor_copy(eT[dit][:, c * 512 : (c + 1) * 512], pst[:])

        transpose_group(eg.pop(c), wr_e, "e")
        project_chunk(
            wk16,
            lambda dit, c=c: eT[dit][:, c * 512 : (c + 1) * 512],
            kT,
            bkt,
            c,
        )
    xl16_pool.release()
    ldE.release()

    # ---- Wv ; V ----
    wv16 = _load_w16(nc, ldW, w16_pool, wv)
    ldW.release()
    ps_tr.release()

    v_pool = tc.alloc_tile_pool(name="v", bufs=NSKV, side="right")
    vt = [v_pool.tile([P, D], BF16, name=f"v{t}", tag="v") for t in range(NSKV)]
    for kt_i in range(NSKV):
        ps_half = []
        for h in range(2):
            psv = ps_proj.tile([P, 512], F32, name=f"psv{h}", tag="psp")
            nc.tensor.matmul(
                psv[:],
                ones_row[:],
                bv16[:, h * 512 : (h + 1) * 512],
                start=True,
                stop=False,
            )
            ps_half.append(psv)
        for dit in range(ND):
            for h in range(2):
                nc.tensor.matmul(
                    ps_half[h][:],
                    eT[dit][:, kt_i * P : (kt_i + 1) * P],
                    wv16[dit][:, h * 512 : (h + 1) * 512],
                    start=False,
                    stop=(dit == ND - 1),
                )
        for h in range(2):
            nc.vector.tensor_copy(vt[kt_i][:, h * 512 : (h + 1) * 512], ps_half[h][:])

    ps_proj.release()
    et_pool.release()
    w16_pool.release()

    # ---- attention: ST+exp phase (full PT materialized), then PV phase ----
    pt_pool = tc.alloc_tile_pool(name="pt", bufs=NSKV, side="right")
    outp = tc.alloc_tile_pool(name="outp", bufs=4, side="right")
    small = tc.alloc_tile_pool(name="small", bufs=4, side="right")

    pT = [pt_pool.tile([P, SQ], BF16, name=f"pT{t}", tag="pT") for t in range(NSKV)]
    ps_st = tc.alloc_tile_pool(name="ps_st", bufs=2, space="PSUM")
    for kt_i in range(NSKV):
        pss = ps_st.tile([P, SQ], F32, name="pss_st", tag="pss_st")
        for dit in range(ND):
            lhsT = kT[dit][:, kt_i * P : (kt_i + 1) * P]
            for c in range(NC):
                nc.tensor.matmul(
                    pss[:, c * 512 : (c + 1) * 512],
                    lhsT,
                    qT[dit][:, c * 512 : (c + 1) * 512],
                    start=(dit == 0),
                    stop=(dit == ND - 1),
                )
        for c in range(NC):
            nc.scalar.activation(
                pT[kt_i][:, c * 512 : (c + 1) * 512],
                pss[:, c * 512 : (c + 1) * 512],
                AF.Exp,
                scale=0.125,
            )
    ps_st.release()

    ps_pv = tc.alloc_tile_pool(name="ps_pv", bufs=2, space="PSUM")
    ps_sum = tc.alloc_tile_pool(name="ps_sum", bufs=2, space="PSUM")
    for sqt in range(NSQ):
        pso = ps_pv.tile([P, D], F32, name="pso", tag="pso")
        psum_s = ps_sum.tile([P, 1], F32, name="psum_s", tag="psum_s")
        for kt_i in range(NSKV):
            lhsT = pT[kt_i][:, sqt * P : (sqt + 1) * P]
            first = kt_i == 0
            last = kt_i == NSKV - 1
            for h in range(2):
                nc.tensor.matmul(
                    pso[:, h * 512 : (h + 1) * 512],
                    lhsT,
                    vt[kt_i][:, h * 512 : (h + 1) * 512],
                    start=first,
                    stop=last,
                )
            nc.tensor.matmul(psum_s[:], lhsT, ones_col[:], start=first, stop=last)
        recip = small.tile([P, 1], F32, name="recip", tag="recip")
        nc.vector.reciprocal(recip[:], psum_s[:])
        ot = outp.tile([P, D], F32, name="ot", tag="ot")
        nc.vector.tensor_scalar_mul(ot[:], pso[:], recip[:])
        nc.sync.dma_start(out.ap()[sqt * P : (sqt + 1) * P, :], ot[:])

    ps_sum.release()
    ps_pv.release()
    small.release()
    outp.release()
    pt_pool.release()
    v_pool.release()
    kt_pool.release()
    qt_pool.release()
    const.release()


_NC_CACHE = []


def _get_nc():
    if not _NC_CACHE:
        _NC_CACHE.append(build())
    return _NC_CACHE[0]


def kernel(
    hidden_states,
    encoder_hidden_states,
    Wq,
    bq,
    Wk,
    bk,
    Wv,
    bv,
    _trace=False,
    _trace_kwargs=None,
):
    hs = np.ascontiguousarray(np.asarray(hidden_states, np.float32))
    es = np.ascontiguousarray(np.asarray(encoder_hidden_states, np.float32))
    wq_ = np.ascontiguousarray(np.asarray(Wq, np.float32))
    wk_ = np.ascontiguousarray(np.asarray(Wk, np.float32))
    wv_ = np.ascontiguousarray(np.asarray(Wv, np.float32))
    bq_ = np.ascontiguousarray(np.asarray(bq, np.float32))
    bk_ = np.ascontiguousarray(np.asarray(bk, np.float32))
    bv_ = np.ascontiguousarray(np.asarray(bv, np.float32))

    nc = _get_nc()
    in_maps = [
        {
            "x": hs[c],
            "e": es[c],
            "wq": wq_,
            "wk": wk_,
            "wv": wv_,
            "bq": bq_,
            "bk": bk_,
            "bv": bv_,
        }
        for c in range(N_CORES)
    ]
    res = run_bass_kernel_spmd(
        nc,
        in_maps,
        list(range(N_CORES)),
        trace=_trace,
        **(_trace_kwargs or {}),
    )
    out = np.stack([res.results[c]["out"] for c in range(N_CORES)], axis=0)
    if _trace:
        return out, res
    return out
